# revision 14
# baseline (speedup 1.0000x reference)
"""Trainium2 Bass kernel for MineralFusion (dwconv fusion + topk masking + SE).

Self-contained: shards batch across 8 NeuronCores (data parallel), runs a
Bass/Tile kernel per core via run_bass_kernel_spmd, gathers full output.
"""
import os
import numpy as np
import ml_dtypes

B, C, H, W = 32, 256, 56, 56
K = 30
N_CORES = 8
B_LOC = B // N_CORES          # 4 samples per core
NBLK = C // 128               # 2 channel blocks per sample
NTILES = B_LOC * NBLK         # 8 tiles per core

PW = 62                       # padded plane side (56 + 3 + 3)
PLANE = PW * PW               # 3844
ORIG = 3 * PW + 3             # interior origin offset (row 3, col 3)
CMP = 3584                    # compact chunked size: 7 chunks x 512
PLANE_X = PLANE + 8           # xbf slack so 8x62 matmul windows stay in range
NEG_BIG = -(2.0 ** 100)       # match_replace sentinel (exact in f32 and bf16)
POS_BIG = +(2.0 ** 100)

TAPS5 = [(dy, dx) for dy in range(-2, 3) for dx in range(-2, 3)]
TAPS3 = [(dy, dx) for dy in range(-1, 2) for dx in range(-1, 2)]
TAPS7 = [(dy, dx) for dy in range(-3, 4) for dx in range(-3, 4)]

# engine split: (n on DVE) for the fp32 tap chains; rest on GPSIMD
FUSED_DVE = 17
SCORE_DVE = 6

LAST = {}


def _pad_view(ap_flat, dy, dx):
    """4-dim interior view [128, 7, 8, 56] of a padded [128, PLANE] tile,
    shifted by tap (dy, dx)."""
    off = ORIG + dy * PW + dx
    v = ap_flat[:, off:off + 7 * 8 * PW]
    # steps: chunk=8 rows of PW, row=PW, col=1
    return v.rearrange("p (k r w) -> p k r w", k=7, r=8, w=PW)[:, :, :, :56]


def _cmp_view(ap_flat):
    """4-dim data view [128, 7, 8, 56] of a compact [*, CMP] tile."""
    v = ap_flat.rearrange("p (k r) -> p k r", k=7, r=512)[:, :, :448]
    return v.rearrange("p k (r w) -> p k r w", r=8, w=56)


def build_nc(a_val):
    import concourse.bass as bass
    import concourse.mybir as mybir
    from concourse import bacc, tile

    f32 = mybir.dt.float32
    bf16 = mybir.dt.bfloat16
    AF = mybir.ActivationFunctionType
    OP = mybir.AluOpType

    nc = bacc.Bacc("TRN2", target_bir_lowering=False, debug=False)

    x_d = nc.declare_dram_parameter("x", [B_LOC, C, H, W], f32, isOutput=False)
    wf_d = nc.declare_dram_parameter("wf", [NBLK, 128, 25], f32, isOutput=False)
    bf_d = nc.declare_dram_parameter("bf", [NBLK, 128, 1], f32, isOutput=False)
    ws_d = nc.declare_dram_parameter("wsc", [NBLK, 128, 9], f32, isOutput=False)
    bs_d = nc.declare_dram_parameter("bsc", [NBLK, 128, 1], f32, isOutput=False)
    dg_d = nc.declare_dram_parameter("diag", [NBLK, 128, 49 * 128], bf16, isOutput=False)
    b3_d = nc.declare_dram_parameter("b3p", [NBLK, 128, 1], f32, isOutput=False)
    s1_d = nc.declare_dram_parameter("sew1", [NBLK, 128, 16], f32, isOutput=False)
    s2_d = nc.declare_dram_parameter("sew2", [NBLK, 16, 128], f32, isOutput=False)
    out_d = nc.declare_dram_parameter("out", [B_LOC, C, H, W], f32, isOutput=True)

    a = float(a_val)

    with tile.TileContext(nc) as tc:
        with (
            tc.tile_pool(name="wpool", bufs=1) as wpool,
            tc.tile_pool(name="xp", bufs=2) as xp_pool,
            tc.tile_pool(name="xbf", bufs=2) as xbf_pool,
            tc.tile_pool(name="fus", bufs=2) as fus_pool,
            tc.tile_pool(name="gpart", bufs=2) as gp_pool,
            tc.tile_pool(name="scr", bufs=2) as scr_pool,
            tc.tile_pool(name="y0", bufs=3) as y0_pool,
            tc.tile_pool(name="small", bufs=12) as sm_pool,
            tc.tile_pool(name="gs", bufs=5) as gs_pool,
            tc.tile_pool(name="gate", bufs=4) as gate_pool,
            tc.tile_pool(name="hsb", bufs=2) as hsb_pool,
            tc.tile_pool(name="c3p", bufs=1, space="PSUM") as c3p_pool,
            tc.tile_pool(name="sep", bufs=1, space="PSUM") as sep_pool,
        ):
            # ---- preload weights ----
            wf_sb = wpool.tile([128, NBLK * 25], f32)
            bf_sb = wpool.tile([128, NBLK], f32)
            ws_sb = wpool.tile([128, NBLK * 9], f32)
            bs_sb = wpool.tile([128, NBLK], f32)
            dg_sb = wpool.tile([128, NBLK * 49 * 128], bf16)
            b3_sb = wpool.tile([128, NBLK], f32)
            s1_sb = wpool.tile([128, NBLK * 16], f32)
            s2_sb = wpool.tile([16, NBLK * 128], f32)
            for blk in range(NBLK):
                nc.sync.dma_start(out=wf_sb[:, blk * 25:(blk + 1) * 25], in_=wf_d[blk])
                nc.sync.dma_start(out=bf_sb[:, blk:blk + 1], in_=bf_d[blk])
                nc.sync.dma_start(out=ws_sb[:, blk * 9:(blk + 1) * 9], in_=ws_d[blk])
                nc.sync.dma_start(out=bs_sb[:, blk:blk + 1], in_=bs_d[blk])
                nc.sync.dma_start(out=dg_sb[:, blk * 6272:(blk + 1) * 6272], in_=dg_d[blk])
                nc.sync.dma_start(out=b3_sb[:, blk:blk + 1], in_=b3_d[blk])
                nc.sync.dma_start(out=s1_sb[:, blk * 16:(blk + 1) * 16], in_=s1_d[blk])
                nc.sync.dma_start(out=s2_sb[:, blk * 128:(blk + 1) * 128], in_=s2_d[blk])

            gsums = {}
            y0s = {}

            for t in range(NTILES):
                b, blk = divmod(t, NBLK)
                c0 = blk * 128

                xp = xp_pool.tile([128, PLANE], f32)
                # zero the pad ring (interior overwritten by DMA)
                nc.gpsimd.memset(xp[:, 0:3 * PW], 0.0)
                nc.gpsimd.memset(xp[:, 59 * PW:PLANE], 0.0)
                lcol = xp[:, 3 * PW:59 * PW].rearrange("p (h w) -> p h w", w=PW)
                nc.gpsimd.memset(lcol[:, :, 0:3], 0.0)
                nc.gpsimd.memset(lcol[:, :, 59:62], 0.0)

                x_src = x_d[b, c0:c0 + 128].rearrange("c h w -> c (h w)") \
                    .rearrange("c (k r w) -> c k r w", k=7, r=8, w=56)
                nc.sync.dma_start(out=_pad_view(xp, 0, 0), in_=x_src)

                # bf16 copy of the padded plane for the PE conv
                xbf = xbf_pool.tile([128, PLANE_X], bf16)
                nc.gpsimd.memset(xbf[:, PLANE:PLANE_X], 0.0)
                nc.scalar.activation(xbf[:, 0:PLANE], xp[:], AF.Copy)

                # ---- fused = conv5x5(x, w12) + b12 (fp32) ----
                fus = fus_pool.tile([128, PLANE], f32)
                nc.gpsimd.memset(fus[:, 0:3 * PW], 0.0)
                nc.gpsimd.memset(fus[:, 59 * PW:PLANE], 0.0)
                fcol = fus[:, 3 * PW:59 * PW].rearrange("p (h w) -> p h w", w=PW)
                nc.gpsimd.memset(fcol[:, :, 0:3], 0.0)
                nc.gpsimd.memset(fcol[:, :, 59:62], 0.0)

                fint = _pad_view(fus, 0, 0)
                dy, dx = TAPS5[0]
                nc.vector.tensor_scalar(
                    fint, _pad_view(xp, dy, dx),
                    wf_sb[:, blk * 25:blk * 25 + 1], bf_sb[:, blk:blk + 1],
                    OP.mult, OP.add)
                for i in range(1, 25):
                    dy, dx = TAPS5[i]
                    nc.vector.scalar_tensor_tensor(
                        fint, _pad_view(xp, dy, dx),
                        wf_sb[:, blk * 25 + i:blk * 25 + i + 1], fint,
                        OP.mult, OP.add)

                # ---- scores = conv3x3(fused, ws) + bs (fp32, compact) ----
                scr = scr_pool.tile([128, CMP], f32)
                sint = _cmp_view(scr[:])
                dy, dx = TAPS3[0]
                nc.vector.tensor_scalar(
                    sint, _pad_view(fus, dy, dx),
                    ws_sb[:, blk * 9:blk * 9 + 1], bs_sb[:, blk:blk + 1],
                    OP.mult, OP.add)
                for i in range(1, 9):
                    dy, dx = TAPS3[i]
                    nc.vector.scalar_tensor_tensor(
                        sint, _pad_view(fus, dy, dx),
                        ws_sb[:, blk * 9 + i:blk * 9 + i + 1], sint,
                        OP.mult, OP.add)
                # junk columns (448:512 of each chunk) must lose the topk
                scr3 = scr[:].rearrange("p (k r) -> p k r", k=7, r=512)
                nc.gpsimd.memset(scr3[:, :, 448:512], NEG_BIG)

                # ---- c3' = conv7x7(x, (1-a)w3) + (1-a)b3 via PE diag matmuls ----
                c3p = c3p_pool.tile([128, CMP], f32)
                for ti, (dy, dx) in enumerate(TAPS7):
                    lhs = dg_sb[:, blk * 6272 + ti * 128: blk * 6272 + (ti + 1) * 128]
                    for ch in range(7):
                        off = ORIG + dy * PW + dx + ch * 8 * PW
                        rhs = xbf[:, off:off + 8 * PW] \
                            .rearrange("p (r w) -> p r w", r=8, w=PW)[:, :, :56]
                        dst = c3p[:, ch * 512: ch * 512 + 448] \
                            .rearrange("p (r w) -> p r w", r=8, w=56)
                        nc.tensor.matmul(dst, lhs, rhs,
                                         start=(ti == 0), stop=(ti == 48))

                # ---- topk: extract top-30 via max + match_replace ----
                rv = [sm_pool.tile([128, 8], f32, tag="rv", name=f"rv{t}_{r}")
                      for r in range(4)]
                for r in range(3):
                    nc.vector.max(rv[r][:], scr[:])
                    nc.vector.match_replace(scr[:], rv[r][:], scr[:], NEG_BIG)
                nc.vector.max(rv[3][:], scr[:])
                m4 = sm_pool.tile([128, 8], f32, tag="rv")
                nc.vector.tensor_copy(m4[:, 0:6], rv[3][:, 0:6])
                nc.vector.memset(m4[:, 6:8], POS_BIG)
                nc.vector.match_replace(scr[:], m4[:], scr[:], NEG_BIG)

                # mask = (scr == NEG_BIG); o1 = mask * fused  (in place)
                msk = gp_pool.tile([128, CMP], f32, tag="gpart", name=f"msk{t}")
                mint = _cmp_view(msk[:])
                nc.vector.tensor_scalar(mint, _cmp_view(scr[:]), NEG_BIG, None,
                                        OP.is_equal)
                nc.vector.tensor_mul(mint, mint, fint)

                # ---- y = a*o1 + (x + b3p + c3') ; gsum = sum(y) ----
                y0 = y0_pool.tile([128, CMP], f32)
                yint = _cmp_view(y0[:])
                nc.vector.scalar_tensor_tensor(
                    yint, _pad_view(xp, 0, 0), b3_sb[:, blk:blk + 1],
                    _cmp_view(c3p[:]), OP.add, OP.add)
                gs = gs_pool.tile([128, 1], f32)
                nc.vector.scalar_tensor_tensor(
                    yint, mint, a, yint, OP.mult, OP.add, accum_out=gs[:])
                gsums[t] = gs
                y0s[t] = y0

                # ---- SE + final scale, once both blocks of sample b done ----
                if blk == NBLK - 1:
                    hp = sep_pool.tile([16, 1], f32, tag="sep")
                    for b2 in range(NBLK):
                        nc.tensor.matmul(
                            hp[:], s1_sb[:, b2 * 16:(b2 + 1) * 16],
                            gsums[b * NBLK + b2][:],
                            start=(b2 == 0), stop=(b2 == NBLK - 1))
                    hsb = hsb_pool.tile([16, 1], f32)
                    nc.scalar.activation(hsb[:], hp[:], AF.Relu)
                    for b2 in range(NBLK):
                        glp = sep_pool.tile([128, 1], f32, tag="sep")
                        nc.tensor.matmul(
                            glp[:], s2_sb[:, b2 * 128:(b2 + 1) * 128], hsb[:],
                            start=True, stop=True)
                        gt = gate_pool.tile([128, 1], f32)
                        nc.scalar.activation(gt[:], glp[:], AF.Sigmoid)
                        nc.vector.tensor_scalar_add(gt[:], gt[:], 1.0)
                        t2 = b * NBLK + b2
                        yv = _cmp_view(y0s[t2][:])
                        nc.scalar.activation(yv, yv, AF.Copy, bias=0.0, scale=gt[:])
                        dst = out_d[b, b2 * 128:(b2 + 1) * 128] \
                            .rearrange("c h w -> c (h w)") \
                            .rearrange("c (k r w) -> c k r w", k=7, r=8, w=56)
                        nc.sync.dma_start(out=dst, in_=yv)

    nc.compile()
    return nc


def _host_prep(inputs):
    x = np.ascontiguousarray(inputs["x"], dtype=np.float32)
    w1 = np.asarray(inputs["w1"], dtype=np.float32)
    b1 = np.asarray(inputs["b1"], dtype=np.float32)
    w2 = np.asarray(inputs["w2"], dtype=np.float32)
    b2 = np.asarray(inputs["b2"], dtype=np.float32)
    w3 = np.asarray(inputs["w3"], dtype=np.float32)
    b3 = np.asarray(inputs["b3"], dtype=np.float32)
    ws = np.asarray(inputs["ws"], dtype=np.float32)
    bs = np.asarray(inputs["bs"], dtype=np.float32)
    se_w1 = np.asarray(inputs["se_w1"], dtype=np.float32)
    se_w2 = np.asarray(inputs["se_w2"], dtype=np.float32)
    alpha = float(np.asarray(inputs["alpha"]))

    a = 1.0 / (1.0 + np.exp(-alpha))

    # fused = conv3(x,w1)+b1 + conv5(x,w2)+b2  ->  single 5x5 kernel
    w12 = w2.copy()
    w12[:, :, 1:4, 1:4] += w1
    b12 = b1 + b2
    w3p = (1.0 - a) * w3
    b3p = (1.0 - a) * b3

    wf = w12[:, 0].reshape(C, 25).reshape(NBLK, 128, 25)
    bfv = b12.reshape(NBLK, 128, 1)
    wsc = ws[:, 0].reshape(C, 9).reshape(NBLK, 128, 9)
    bsc = bs.reshape(NBLK, 128, 1)

    d = np.zeros((NBLK, 128, 49, 128), dtype=np.float32)
    for blk in range(NBLK):
        for c in range(128):
            d[blk, c, :, c] = w3p[blk * 128 + c, 0].reshape(49)
    diag = d.reshape(NBLK, 128, 49 * 128).astype(ml_dtypes.bfloat16)

    s1 = (se_w1 / float(H * W)).T.reshape(NBLK, 128, 16)  # [c, j] blocks
    s2 = se_w2.T.reshape(16, NBLK, 128).transpose(1, 0, 2)  # [blk, j, m]

    common = {
        "wf": np.ascontiguousarray(wf, np.float32),
        "bf": np.ascontiguousarray(bfv, np.float32),
        "wsc": np.ascontiguousarray(wsc, np.float32),
        "bsc": np.ascontiguousarray(bsc, np.float32),
        "diag": np.ascontiguousarray(diag),
        "b3p": np.ascontiguousarray(b3p.reshape(NBLK, 128, 1), np.float32),
        "sew1": np.ascontiguousarray(s1, np.float32),
        "sew2": np.ascontiguousarray(s2, np.float32),
    }
    return x, a, common


def kernel(**inputs):
    from concourse.bass_utils import run_bass_kernel_spmd

    x, a, common = _host_prep(inputs)
    nc = build_nc(a)

    in_maps = []
    for i in range(N_CORES):
        m = {"x": np.ascontiguousarray(x[i * B_LOC:(i + 1) * B_LOC])}
        m.update(common)
        in_maps.append(m)

    res = run_bass_kernel_spmd(nc, in_maps, core_ids=list(range(N_CORES)))
    LAST.clear()
    LAST["exec_time_ns"] = res.exec_time_ns
    LAST["mean_exec_time_ns"] = res.mean_exec_time_ns
    out = np.concatenate([res.results[i]["out"] for i in range(N_CORES)], axis=0)
    return out
